# revision 7
# baseline (speedup 1.0000x reference)
"""DeepseekV3 MoE (E=16, K=4, H=1024, I=512, shared 2x) on 8 trn2 NeuronCores.

Expert-parallel: 2 routed experts per core (host gathers each expert's tokens),
shared expert + residual data-parallel over 512-token slices. Host does the
gate (fp32 numpy, reference-exact) and the token all-to-all (gather/scatter);
all matmuls/activations run on-device in bf16 with fp32 accumulation.

v2: weight-stationary G/U matmuls produce G^T/U^T [I-part, tok] directly in
PSUM, so the down-proj needs no PE transposes and no act copies; the ACT
engine runs only Sigmoid (no table thrash); combine weights are folded into
the down-proj PSUM eviction on DVE; DMA pieces are contiguous SBUF ranges
ordered so the PE starts ~2us in and never waits on weights.
"""

import os
import sys
import types
import numpy as np
import ml_dtypes

import concourse.bass as bass
import concourse.mybir as mybir
import concourse.tile as tile
from concourse import bacc
from concourse.bass_utils import run_bass_kernel_spmd

BF16 = mybir.dt.bfloat16
F32 = mybir.dt.float32
NP_BF16 = ml_dtypes.bfloat16

E, K, NG, TG = 16, 4, 4, 2
SCALE = 2.5
H, I, SH_I = 1024, 512, 1024
B, S = 2, 2048
N = B * S
NCORES = 8
EPC = E // NCORES          # experts per core = 2
NSH = N // NCORES          # shared-expert tokens per core = 512
HC = H // 128              # 8 h-chunks
IC = I // 128              # 4 i-chunks (routed)
SIC = SH_I // 128          # 8 i-chunks (shared)
NQ = 4                     # shared i-quarters (2 chunks each)
MIN_CE = 1152              # per-expert token capacity (multiple of 128)
GB = 256                   # routed token-group/block width


def _gate_cw(xf: np.ndarray, gate_w: np.ndarray, gate_bias: np.ndarray) -> np.ndarray:
    """Reference-exact MoE gate in numpy fp32. Returns cw [N, E]."""
    logits = xf @ gate_w.T
    scores = 1.0 / (1.0 + np.exp(-logits))
    sfc = scores + gate_bias
    epg = E // NG
    grp = sfc.reshape(N, NG, epg)
    top2 = np.sort(grp, axis=-1)[:, :, -2:].sum(-1)
    gidx = np.argsort(-top2, axis=1, kind="stable")[:, :TG]
    gmask = np.zeros((N, NG), bool)
    np.put_along_axis(gmask, gidx, True, axis=1)
    emask = np.repeat(gmask, epg, axis=1)
    masked = np.where(emask, sfc, -np.inf)
    topk_idx = np.argsort(-masked, axis=1, kind="stable")[:, :K]
    topk_w = np.take_along_axis(scores, topk_idx, axis=1)
    topk_w = topk_w / (topk_w.sum(-1, keepdims=True) + 1e-20)
    topk_w = topk_w * SCALE
    cw = np.zeros((N, E), np.float32)
    np.put_along_axis(cw, topk_idx, topk_w.astype(np.float32), axis=1)
    return cw


def _blocks(cea: int, ceb: int):
    """Token-group blocks tiling the [m] slot space: (slot, col0, n, tt0)."""
    out = []
    tt = 0
    for e, (base, cap) in enumerate(((0, cea), (cea, ceb))):
        off = 0
        while off < cap:
            n = min(GB, cap - off)
            out.append((e, base + off, n, tt))
            tt += n // 128
            off += n
    return out


_BUILD_CACHE: dict[tuple, object] = {}


def _build(cea: int, ceb: int):
    """Build + compile the per-core SPMD Tile program."""
    key = (cea, ceb)
    if key in _BUILD_CACHE:
        return _BUILD_CACHE[key]
    blocks = _blocks(cea, ceb)
    nb = len(blocks)
    tt_total = (cea + ceb) // 128
    m = cea + ceb

    nc = bacc.Bacc("TRN2", target_bir_lowering=False, debug=False,
                   num_devices=NCORES)
    xg_t = nc.dram_tensor("xg_t", [nb, H, GB], BF16, kind="ExternalInput").ap()
    cw_pt = nc.dram_tensor("cw_pt", [128, tt_total], F32, kind="ExternalInput").ap()
    wg_t = nc.dram_tensor("wg_t", [EPC, H, I], BF16, kind="ExternalInput").ap()
    wu_t = nc.dram_tensor("wu_t", [EPC, H, I], BF16, kind="ExternalInput").ap()
    wd_t = nc.dram_tensor("wd_t", [EPC, I, H], BF16, kind="ExternalInput").ap()
    xs_t = nc.dram_tensor("xs_t", [H, NSH], BF16, kind="ExternalInput").ap()
    xres = nc.dram_tensor("xres", [NSH, H], BF16, kind="ExternalInput").ap()
    wsg_t = nc.dram_tensor("wsg_t", [H, SH_I], BF16, kind="ExternalInput").ap()
    wsu_t = nc.dram_tensor("wsu_t", [H, SH_I], BF16, kind="ExternalInput").ap()
    wsd_t = nc.dram_tensor("wsd_t", [SH_I, H], BF16, kind="ExternalInput").ap()
    yg = nc.dram_tensor("yg", [m, H], BF16, kind="ExternalOutput").ap()
    ybase = nc.dram_tensor("ybase", [NSH, H], BF16, kind="ExternalOutput").ap()

    SIGM = mybir.ActivationFunctionType.Sigmoid
    COPY = mybir.ActivationFunctionType.Copy

    with tile.TileContext(nc) as tc:
        with (
            tc.tile_pool(name="const", bufs=1) as const,
            tc.tile_pool(name="sb_act", bufs=6) as sb_act,
            tc.tile_pool(name="act_rt", bufs=2) as act_rt,
            tc.tile_pool(name="sb_out", bufs=3) as sb_out,
            tc.tile_pool(name="ps_gu", bufs=6, space=bass.MemorySpace.PSUM) as ps_gu,
            tc.tile_pool(name="ps_y", bufs=2, space=bass.MemorySpace.PSUM) as ps_y,
        ):
            # ---- resident SBUF tiles; every DMA piece is a contiguous SBUF
            # byte range (Tile tracks DMA->compute deps by bounding box), in
            # the order compute consumes them ----
            wsg_sb = const.tile([128, NQ, HC, 256], BF16, tag="wsg")
            wsu_sb = const.tile([128, NQ, HC, 256], BF16, tag="wsu")
            xs_sb = const.tile([128, HC, NSH], BF16, tag="xs")
            wsd_sb = const.tile([128, SIC, H], BF16, tag="wsd")
            xres_sb = const.tile([128, NSH // 128, H], BF16, tag="xres")
            cw_sb = const.tile([128, tt_total], F32, tag="cw")
            wg_sb = const.tile([128, EPC, HC, I], BF16, tag="wg")
            wu_sb = const.tile([128, EPC, HC, I], BF16, tag="wu")
            wd_sb = const.tile([128, EPC, IC, H], BF16, tag="wd")
            xg_sb = const.tile([128, nb, HC, GB], BF16, tag="xg")
            act_sh = const.tile([128, SIC, NSH], BF16, tag="act_sh")

            wsg_r = wsg_t.rearrange("(c p) (q i) -> p q c i", p=128, q=NQ)
            wsu_r = wsu_t.rearrange("(c p) (q i) -> p q c i", p=128, q=NQ)
            xs_r = xs_t.rearrange("(c p) n -> p c n", p=128)
            wsd_r = wsd_t.rearrange("(c p) h -> p c h", p=128)
            xres_r = xres.rearrange("(t p) h -> p t h", p=128)
            wg_r = wg_t.rearrange("e (c p) i -> p e c i", p=128)
            wu_r = wu_t.rearrange("e (c p) i -> p e c i", p=128)
            wd_r = wd_t.rearrange("e (c p) h -> p e c h", p=128)
            xg_r = xg_t.rearrange("b (c p) n -> p b c n", p=128)

            nc.sync.dma_start(wsg_sb[:, 0], wsg_r[:, 0])
            nc.sync.dma_start(xs_sb[:, 0:4, :], xs_r[:, 0:4, :])
            nc.sync.dma_start(xs_sb[:, 4:8, :], xs_r[:, 4:8, :])
            nc.sync.dma_start(wsu_sb[:, 0], wsu_r[:, 0])
            for q in range(1, NQ):
                nc.sync.dma_start(wsg_sb[:, q], wsg_r[:, q])
                nc.sync.dma_start(wsu_sb[:, q], wsu_r[:, q])
            nc.sync.dma_start(cw_sb[:], cw_pt[:])
            nc.sync.dma_start(wg_sb[:, 0], wg_r[:, 0])
            nc.sync.dma_start(wu_sb[:, 0], wu_r[:, 0])
            nc.sync.dma_start(xg_sb[:, 0], xg_r[:, 0])
            nc.sync.dma_start(wsd_sb[:, 0:4, :], wsd_r[:, 0:4, :])
            nc.sync.dma_start(wsd_sb[:, 4:8, :], wsd_r[:, 4:8, :])
            nc.sync.dma_start(xres_sb[:], xres_r[:])
            nc.sync.dma_start(wd_sb[:, 0], wd_r[:, 0])
            for b in range(1, 3):
                nc.sync.dma_start(xg_sb[:, b], xg_r[:, b])
            nc.sync.dma_start(wg_sb[:, 1], wg_r[:, 1])
            nc.sync.dma_start(wu_sb[:, 1], wu_r[:, 1])
            for b in range(3, nb):
                nc.sync.dma_start(xg_sb[:, b], xg_r[:, b])
            nc.sync.dma_start(wd_sb[:, 1], wd_r[:, 1])

            # ---- shared expert G^T/U^T: out [I-local 128, tok 512] ----
            for q in range(NQ):
                g_ps, u_ps = [], []
                for w_sb, dst in ((wsg_sb, g_ps), (wsu_sb, u_ps)):
                    for i in range(2):
                        t_ps = ps_gu.tile([128, NSH], F32, tag="gu")
                        for c in range(HC):
                            nc.tensor.matmul(t_ps[:],
                                             w_sb[:, q, c, i * 128:(i + 1) * 128],
                                             xs_sb[:, c, :],
                                             start=(c == 0), stop=(c == HC - 1))
                        dst.append(t_ps)
                for i in range(2):
                    p_sb = sb_act.tile([128, NSH], BF16, tag="p")
                    nc.scalar.activation(p_sb[:], g_ps[i][:], SIGM)
                    t_sb = sb_act.tile([128, NSH], BF16, tag="t")
                    nc.vector.tensor_mul(t_sb[:], p_sb[:], g_ps[i][:])
                    nc.vector.tensor_mul(act_sh[:, 2 * q + i, :], t_sb[:],
                                         u_ps[i][:])

            def routed_gu(blk):
                e, col0, n, tt0, b = blk
                gus = []
                for w_sb in (wg_sb, wu_sb):
                    for pr in range(2):
                        t_ps = ps_gu.tile([128, 2, n], F32, tag="gu")
                        for ii in range(2):
                            ci = 2 * pr + ii
                            for c in range(HC):
                                nc.tensor.matmul(
                                    t_ps[:, ii, :],
                                    w_sb[:, e, c, ci * 128:(ci + 1) * 128],
                                    xg_sb[:, b, c, 0:n],
                                    start=(c == 0), stop=(c == HC - 1))
                        gus.append(t_ps)
                return gus

            def routed_down(blk, gus):
                e, col0, n, tt0, b = blk
                act = act_rt.tile([128, IC, n], BF16, tag="act")
                for pr in range(2):
                    g_ps, u_ps = gus[pr], gus[2 + pr]
                    p_sb = sb_act.tile([128, 2, n], BF16, tag="p")
                    nc.scalar.activation(p_sb[:], g_ps[:], SIGM)
                    t_sb = sb_act.tile([128, 2, n], BF16, tag="t")
                    nc.vector.tensor_mul(t_sb[:], p_sb[:], g_ps[:])
                    nc.vector.tensor_mul(act[:, 2 * pr:2 * pr + 2, :], t_sb[:],
                                         u_ps[:])
                last = (col0 + n == cea + ceb)
                for t in range(n // 128):
                    tt = tt0 + t
                    y_sb = sb_out.tile([128, H], BF16, tag="y")
                    for hh in range(2):
                        y_ps = ps_y.tile([128, 512], F32, tag="y_ps")
                        for ci in range(IC):
                            nc.tensor.matmul(
                                y_ps[:], act[:, ci, t * 128:(t + 1) * 128],
                                wd_sb[:, e, ci, hh * 512:(hh + 1) * 512],
                                start=(ci == 0), stop=(ci == IC - 1))
                        # evictions alternate ACT/DVE so neither engine's
                        # backlog gates PSUM-bank reuse
                        if hh == 0:
                            nc.scalar.activation(
                                y_sb[:, 0:512], y_ps[:], COPY,
                                scale=cw_sb[:, tt:tt + 1])
                        else:
                            nc.vector.tensor_scalar_mul(
                                y_sb[:, 512:1024], y_ps[:],
                                cw_sb[:, tt:tt + 1])
                        if last:
                            nc.sync.dma_start(
                                yg[col0 + t * 128:col0 + (t + 1) * 128,
                                   hh * 512:(hh + 1) * 512],
                                y_sb[:, hh * 512:(hh + 1) * 512])
                    if not last:
                        nc.sync.dma_start(
                            yg[col0 + t * 128:col0 + (t + 1) * 128, :], y_sb[:])

            # routed block 0 G/U fills the PE while shared act finishes
            blk0 = blocks[0] + (0,)
            pend = (blk0, routed_gu(blk0))

            # ---- shared down + residual ----
            for t in range(NSH // 128):
                ob = sb_out.tile([128, H], BF16, tag="ob")
                for hh in range(2):
                    y2 = ps_y.tile([128, 512], F32, tag="y_ps")
                    for ci in range(SIC):
                        nc.tensor.matmul(
                            y2[:], act_sh[:, ci, t * 128:(t + 1) * 128],
                            wsd_sb[:, ci, hh * 512:(hh + 1) * 512],
                            start=(ci == 0), stop=(ci == SIC - 1))
                    nc.vector.tensor_add(ob[:, hh * 512:(hh + 1) * 512], y2[:],
                                         xres_sb[:, t, hh * 512:(hh + 1) * 512])
                nc.sync.dma_start(ybase[t * 128:(t + 1) * 128, :], ob[:])

            # ---- routed blocks, 2-stage software pipeline ----
            for bi in range(1, len(blocks)):
                blk = blocks[bi] + (bi,)
                gus = routed_gu(blk)
                routed_down(*pend)
                pend = (blk, gus)
            routed_down(*pend)

    nc.compile()
    _BUILD_CACHE[key] = nc
    return nc


def _prepare(inputs: dict, caps, pairs, cw: np.ndarray, idx: list[np.ndarray]):
    """Build per-core input maps. idx[e] = token indices routed to expert e."""
    cea, ceb = caps
    blocks = _blocks(cea, ceb)
    nb = len(blocks)
    xf = np.asarray(inputs["hidden_states"], np.float32).reshape(N, H)
    xt_bf = np.ascontiguousarray(xf.T).astype(NP_BF16)        # [H, N]
    wg = np.asarray(inputs["Wg"])
    wu = np.asarray(inputs["Wu"])
    wd = np.asarray(inputs["Wd"])
    wsg = np.asarray(inputs["Ws_g"])
    wsu = np.asarray(inputs["Ws_u"])
    wsd = np.asarray(inputs["Ws_d"])
    bases = [0, cea]
    m = cea + ceb
    wsg_bf = np.ascontiguousarray(wsg.T).astype(NP_BF16)
    wsu_bf = np.ascontiguousarray(wsu.T).astype(NP_BF16)
    wsd_bf = np.ascontiguousarray(wsd.T).astype(NP_BF16)
    in_maps = []
    for core in range(NCORES):
        es = pairs[core]
        xg = np.zeros((H, m), NP_BF16)
        cwg = np.zeros((m,), np.float32)
        for j, e in enumerate(es):
            ne = len(idx[e])
            xg[:, bases[j]:bases[j] + ne] = xt_bf[:, idx[e]]
            cwg[bases[j]:bases[j] + ne] = cw[idx[e], e]
        xgb = np.zeros((nb, H, GB), NP_BF16)
        for i, (_, col0, n, _) in enumerate(blocks):
            xgb[i, :, :n] = xg[:, col0:col0 + n]
        sl = slice(core * NSH, (core + 1) * NSH)
        in_maps.append({
            "xg_t": xgb,
            "cw_pt": np.ascontiguousarray(cwg.reshape(-1, 128).T),
            "wg_t": np.ascontiguousarray(
                wg[list(es)].transpose(0, 2, 1)).astype(NP_BF16),
            "wu_t": np.ascontiguousarray(
                wu[list(es)].transpose(0, 2, 1)).astype(NP_BF16),
            "wd_t": np.ascontiguousarray(
                wd[list(es)].transpose(0, 2, 1)).astype(NP_BF16),
            "xs_t": np.ascontiguousarray(xt_bf[:, sl]),
            "xres": np.ascontiguousarray(xf[sl]).astype(NP_BF16),
            "wsg_t": wsg_bf,
            "wsu_t": wsu_bf,
            "wsd_t": wsd_bf,
        })
    return in_maps


def _combine(results, caps, pairs, idx: list[np.ndarray]) -> np.ndarray:
    out = np.empty((N, H), np.float32)
    bases = [0, caps[0]]
    for core in range(NCORES):
        out[core * NSH:(core + 1) * NSH] = np.asarray(
            results[core]["ybase"], np.float32)
    for core in range(NCORES):
        ygr = np.asarray(results[core]["yg"], np.float32)
        for j, e in enumerate(pairs[core]):
            ne = len(idx[e])
            out[idx[e]] += ygr[bases[j]:bases[j] + ne]
    return out.reshape(B, S, H)


def _route(inputs: dict):
    xf = np.asarray(inputs["hidden_states"], np.float32).reshape(N, H)
    cw = _gate_cw(xf, np.asarray(inputs["gate_w"], np.float32),
                  np.asarray(inputs["gate_bias"], np.float32))
    idx = [np.nonzero(cw[:, e])[0] for e in range(E)]
    loads = np.array([len(i) for i in idx])
    order = np.argsort(-loads, kind="stable")
    bigs, smalls = order[:NCORES], order[NCORES:][::-1]
    pairs = [(int(a), int(b)) for a, b in zip(bigs, smalls)]
    cea = max(MIN_CE, -(-int(loads[bigs].max()) // 128) * 128)
    ceb = max(1024, -(-int(loads[smalls].max()) // 128) * 128)
    return cw, idx, (cea, ceb), pairs


def _run(inputs: dict, trace: bool = False, tmpdir: str | None = None):
    cw, idx, caps, pairs = _route(inputs)
    nc = _build(*caps)
    in_maps = _prepare(inputs, caps, pairs, cw, idx)
    res = run_bass_kernel_spmd(nc, in_maps, list(range(NCORES)),
                               trace=trace, tmpdir=tmpdir)
    return _combine(res.results, caps, pairs, idx), res


def kernel(**inputs) -> np.ndarray:
    out, _ = _run(inputs, trace=False)
    return out


def _install_prof_shim():
    """Make run_bass_kernel_spmd(trace=True) work under axon in this image."""
    if "antenv.axon_hooks" in sys.modules:
        return
    try:
        from trn_agent_boot.trn_boot import _ntff_profile_via_ctypes
        hook = _ntff_profile_via_ctypes("/opt/axon/libaxon_pjrt.so")
    except Exception:
        hook = None
    mod = types.ModuleType("antenv.axon_hooks")
    mod.get_axon_ntff_profile_hook = lambda: hook
    mod.set_axon_ntff_profile_hook = lambda h: None
    sys.modules["antenv.axon_hooks"] = mod
    import concourse.bass_utils as bu
    bu.upload_artifacts = lambda tmpdir: tmpdir


def kernel_traced(tmpdir=None, all_cores=False, **inputs):
    """Returns (output, BassKernelResults with exec_time_ns)."""
    _install_prof_shim()
    if all_cores:
        os.environ["BASS_PERFETTO_PROFILE_ALL_CORES"] = "1"
    out, res = _run(inputs, trace=True, tmpdir=tmpdir)
    return out, res


# revision 11
# speedup vs baseline: 1.0206x; 1.0206x over previous
"""DeepseekV3 MoE (E=16, K=4, H=1024, I=512, shared 2x) on 8 trn2 NeuronCores.

Expert-parallel: 2 routed experts per core (host gathers each expert's tokens),
shared expert + residual data-parallel over 512-token slices. Host does the
gate (fp32 numpy, reference-exact) and the token all-to-all (gather/scatter);
all matmuls/activations run on-device in bf16 with fp32 accumulation.

v2: weight-stationary G/U matmuls produce G^T/U^T [I-part, tok] directly in
PSUM, so the down-proj needs no PE transposes and no act copies; the ACT
engine runs only Sigmoid (no table thrash); combine weights are folded into
the down-proj PSUM eviction on DVE; DMA pieces are contiguous SBUF ranges
ordered so the PE starts ~2us in and never waits on weights.
"""

import os
import sys
import types
import numpy as np
import ml_dtypes

import concourse.bass as bass
import concourse.mybir as mybir
import concourse.tile as tile
from concourse import bacc
from concourse.bass_utils import run_bass_kernel_spmd

BF16 = mybir.dt.bfloat16
F32 = mybir.dt.float32
NP_BF16 = ml_dtypes.bfloat16

E, K, NG, TG = 16, 4, 4, 2
SCALE = 2.5
H, I, SH_I = 1024, 512, 1024
B, S = 2, 2048
N = B * S
NCORES = 8
EPC = E // NCORES          # experts per core = 2
NSH = N // NCORES          # shared-expert tokens per core = 512
HC = H // 128              # 8 h-chunks
IC = I // 128              # 4 i-chunks (routed)
SIC = SH_I // 128          # 8 i-chunks (shared)
NQ = 4                     # shared i-quarters (2 chunks each)
MIN_CE = 1152              # per-expert token capacity (multiple of 128)
GB = 256                   # routed token-group/block width


def _gate_cw(xf: np.ndarray, gate_w: np.ndarray, gate_bias: np.ndarray) -> np.ndarray:
    """Reference-exact MoE gate in numpy fp32. Returns cw [N, E]."""
    logits = xf @ gate_w.T
    scores = 1.0 / (1.0 + np.exp(-logits))
    sfc = scores + gate_bias
    epg = E // NG
    grp = sfc.reshape(N, NG, epg)
    top2 = np.sort(grp, axis=-1)[:, :, -2:].sum(-1)
    gidx = np.argsort(-top2, axis=1, kind="stable")[:, :TG]
    gmask = np.zeros((N, NG), bool)
    np.put_along_axis(gmask, gidx, True, axis=1)
    emask = np.repeat(gmask, epg, axis=1)
    masked = np.where(emask, sfc, -np.inf)
    topk_idx = np.argsort(-masked, axis=1, kind="stable")[:, :K]
    topk_w = np.take_along_axis(scores, topk_idx, axis=1)
    topk_w = topk_w / (topk_w.sum(-1, keepdims=True) + 1e-20)
    topk_w = topk_w * SCALE
    cw = np.zeros((N, E), np.float32)
    np.put_along_axis(cw, topk_idx, topk_w.astype(np.float32), axis=1)
    return cw


def _blocks(cea: int, ceb: int):
    """Token-group blocks tiling the [m] slot space: (slot, col0, n, tt0)."""
    out = []
    tt = 0
    for e, (base, cap) in enumerate(((0, cea), (cea, ceb))):
        off = 0
        while off < cap:
            n = min(GB, cap - off)
            out.append((e, base + off, n, tt))
            tt += n // 128
            off += n
    return out


_BUILD_CACHE: dict[tuple, object] = {}


def _build(cea: int, ceb: int):
    """Build + compile the per-core SPMD Tile program."""
    key = (cea, ceb)
    if key in _BUILD_CACHE:
        return _BUILD_CACHE[key]
    blocks = _blocks(cea, ceb)
    nb = len(blocks)
    tt_total = (cea + ceb) // 128
    m = cea + ceb

    nc = bacc.Bacc("TRN2", target_bir_lowering=False, debug=False,
                   num_devices=NCORES)
    xg_t = nc.dram_tensor("xg_t", [nb, H, GB], BF16, kind="ExternalInput").ap()
    cw_pt = nc.dram_tensor("cw_pt", [128, tt_total], F32, kind="ExternalInput").ap()
    wg_t = nc.dram_tensor("wg_t", [EPC, H, I], BF16, kind="ExternalInput").ap()
    wu_t = nc.dram_tensor("wu_t", [EPC, H, I], BF16, kind="ExternalInput").ap()
    wd_t = nc.dram_tensor("wd_t", [EPC, I, H], BF16, kind="ExternalInput").ap()
    xs_t = nc.dram_tensor("xs_t", [H, NSH], BF16, kind="ExternalInput").ap()
    xres = nc.dram_tensor("xres", [NSH, H], BF16, kind="ExternalInput").ap()
    wsg_t = nc.dram_tensor("wsg_t", [H, SH_I], BF16, kind="ExternalInput").ap()
    wsu_t = nc.dram_tensor("wsu_t", [H, SH_I], BF16, kind="ExternalInput").ap()
    wsd_t = nc.dram_tensor("wsd_t", [SH_I, H], BF16, kind="ExternalInput").ap()
    yg = nc.dram_tensor("yg", [m, H], BF16, kind="ExternalOutput").ap()
    ybase = nc.dram_tensor("ybase", [NSH, H], BF16, kind="ExternalOutput").ap()

    SIGM = mybir.ActivationFunctionType.Sigmoid
    COPY = mybir.ActivationFunctionType.Copy

    with tile.TileContext(nc) as tc:
        with (
            tc.tile_pool(name="const", bufs=1) as const,
            tc.tile_pool(name="sb_act", bufs=6) as sb_act,
            tc.tile_pool(name="act_rt", bufs=2) as act_rt,
            tc.tile_pool(name="sb_out", bufs=3) as sb_out,
            tc.tile_pool(name="ps_gu", bufs=5, space=bass.MemorySpace.PSUM) as ps_gu,
            tc.tile_pool(name="ps_y", bufs=3, space=bass.MemorySpace.PSUM) as ps_y,
        ):
            # ---- resident SBUF tiles; every DMA piece is a contiguous SBUF
            # byte range (Tile tracks DMA->compute deps by bounding box), in
            # the order compute consumes them ----
            wsg_sb = const.tile([128, NQ, HC, 256], BF16, tag="wsg")
            wsu_sb = const.tile([128, NQ, HC, 256], BF16, tag="wsu")
            xs_sb = const.tile([128, HC, NSH], BF16, tag="xs")
            wsd_sb = const.tile([128, SIC, H], BF16, tag="wsd")
            xres_sb = const.tile([128, NSH // 128, H], BF16, tag="xres")
            cw_sb = const.tile([128, tt_total], F32, tag="cw")
            wg_sb = const.tile([128, EPC, HC, I], BF16, tag="wg")
            wu_sb = const.tile([128, EPC, HC, I], BF16, tag="wu")
            wd_sb = const.tile([128, EPC, IC, H], BF16, tag="wd")
            xg_sb = const.tile([128, nb, HC, GB], BF16, tag="xg")
            act_sh = const.tile([128, SIC, NSH], BF16, tag="act_sh")

            wsg_r = wsg_t.rearrange("(c p) (q i) -> p q c i", p=128, q=NQ)
            wsu_r = wsu_t.rearrange("(c p) (q i) -> p q c i", p=128, q=NQ)
            xs_r = xs_t.rearrange("(c p) n -> p c n", p=128)
            wsd_r = wsd_t.rearrange("(c p) h -> p c h", p=128)
            xres_r = xres.rearrange("(t p) h -> p t h", p=128)
            wg_r = wg_t.rearrange("e (c p) i -> p e c i", p=128)
            wu_r = wu_t.rearrange("e (c p) i -> p e c i", p=128)
            wd_r = wd_t.rearrange("e (c p) h -> p e c h", p=128)
            xg_r = xg_t.rearrange("b (c p) n -> p b c n", p=128)

            # startup triggers spread across engine queues so the 4 first
            # DMAs issue in ~1.2us instead of serializing on SP
            nc.sync.dma_start(wsg_sb[:, 0], wsg_r[:, 0])
            nc.gpsimd.dma_start(xs_sb[:, 0:4, :], xs_r[:, 0:4, :])
            nc.gpsimd.dma_start(xs_sb[:, 4:8, :], xs_r[:, 4:8, :])
            nc.scalar.dma_start(wsu_sb[:, 0], wsu_r[:, 0])
            for q in range(1, NQ):
                nc.sync.dma_start(wsg_sb[:, q], wsg_r[:, q])
                nc.sync.dma_start(wsu_sb[:, q], wsu_r[:, q])
            nc.sync.dma_start(cw_sb[:], cw_pt[:])
            nc.sync.dma_start(wg_sb[:, 0], wg_r[:, 0])
            nc.sync.dma_start(wu_sb[:, 0], wu_r[:, 0])
            nc.sync.dma_start(xg_sb[:, 0], xg_r[:, 0])
            nc.sync.dma_start(wsd_sb[:, 0:4, :], wsd_r[:, 0:4, :])
            nc.sync.dma_start(wsd_sb[:, 4:8, :], wsd_r[:, 4:8, :])
            nc.sync.dma_start(xres_sb[:], xres_r[:])
            nc.sync.dma_start(wd_sb[:, 0], wd_r[:, 0])
            for b in range(1, 3):
                nc.sync.dma_start(xg_sb[:, b], xg_r[:, b])
            nc.sync.dma_start(wg_sb[:, 1], wg_r[:, 1])
            nc.sync.dma_start(wu_sb[:, 1], wu_r[:, 1])
            for b in range(3, nb):
                nc.sync.dma_start(xg_sb[:, b], xg_r[:, b])
            nc.sync.dma_start(wd_sb[:, 1], wd_r[:, 1])

            # ---- shared expert G^T/U^T: out [I-local 128, tok 512] ----
            for q in range(NQ):
                g_ps, u_ps = [], []
                for w_sb, dst in ((wsg_sb, g_ps), (wsu_sb, u_ps)):
                    for i in range(2):
                        t_ps = ps_gu.tile([128, NSH], F32, tag="gu")
                        for c in range(HC):
                            nc.tensor.matmul(t_ps[:],
                                             w_sb[:, q, c, i * 128:(i + 1) * 128],
                                             xs_sb[:, c, :],
                                             start=(c == 0), stop=(c == HC - 1))
                        dst.append(t_ps)
                for i in range(2):
                    p_sb = sb_act.tile([128, NSH], BF16, tag="p")
                    nc.scalar.activation(p_sb[:], g_ps[i][:], SIGM)
                    t_sb = sb_act.tile([128, NSH], BF16, tag="t")
                    nc.vector.tensor_mul(t_sb[:], p_sb[:], g_ps[i][:])
                    nc.vector.tensor_mul(act_sh[:, 2 * q + i, :], t_sb[:],
                                         u_ps[i][:])

            def routed_gu(blk):
                e, col0, n, tt0, b = blk
                gus = []
                for w_sb in (wg_sb, wu_sb):
                    for pr in range(2):
                        t_ps = ps_gu.tile([128, 2, n], F32, tag="gu")
                        for ii in range(2):
                            ci = 2 * pr + ii
                            for c in range(HC):
                                nc.tensor.matmul(
                                    t_ps[:, ii, :],
                                    w_sb[:, e, c, ci * 128:(ci + 1) * 128],
                                    xg_sb[:, b, c, 0:n],
                                    start=(c == 0), stop=(c == HC - 1))
                        gus.append(t_ps)
                return gus

            def routed_down(blk, gus):
                e, col0, n, tt0, b = blk
                act = act_rt.tile([128, IC, n], BF16, tag="act")
                for pr in range(2):
                    g_ps, u_ps = gus[pr], gus[2 + pr]
                    p_sb = sb_act.tile([128, 2, n], BF16, tag="p")
                    nc.scalar.activation(p_sb[:], g_ps[:], SIGM)
                    t_sb = sb_act.tile([128, 2, n], BF16, tag="t")
                    nc.vector.tensor_mul(t_sb[:], p_sb[:], g_ps[:])
                    nc.vector.tensor_mul(act[:, 2 * pr:2 * pr + 2, :], t_sb[:],
                                         u_ps[:])
                last = (col0 + n == cea + ceb)
                for t in range(n // 128):
                    tt = tt0 + t
                    y_sb = sb_out.tile([128, H], BF16, tag="y")
                    for hh in range(2):
                        y_ps = ps_y.tile([128, 512], F32, tag="y_ps")
                        for ci in range(IC):
                            nc.tensor.matmul(
                                y_ps[:], act[:, ci, t * 128:(t + 1) * 128],
                                wd_sb[:, e, ci, hh * 512:(hh + 1) * 512],
                                start=(ci == 0), stop=(ci == IC - 1))
                        # evictions alternate ACT/DVE so neither engine's
                        # backlog gates PSUM-bank reuse
                        if hh == 0:
                            nc.scalar.activation(
                                y_sb[:, 0:512], y_ps[:], COPY,
                                scale=cw_sb[:, tt:tt + 1])
                        else:
                            nc.vector.tensor_scalar_mul(
                                y_sb[:, 512:1024], y_ps[:],
                                cw_sb[:, tt:tt + 1])
                        if last:
                            nc.sync.dma_start(
                                yg[col0 + t * 128:col0 + (t + 1) * 128,
                                   hh * 512:(hh + 1) * 512],
                                y_sb[:, hh * 512:(hh + 1) * 512])
                    if not last:
                        nc.sync.dma_start(
                            yg[col0 + t * 128:col0 + (t + 1) * 128, :], y_sb[:])

            def shared_down(t):
                ob = sb_out.tile([128, H], BF16, tag="ob")
                for hh in range(2):
                    y2 = ps_y.tile([128, 512], F32, tag="y_ps")
                    for ci in range(SIC):
                        nc.tensor.matmul(
                            y2[:], act_sh[:, ci, t * 128:(t + 1) * 128],
                            wsd_sb[:, ci, hh * 512:(hh + 1) * 512],
                            start=(ci == 0), stop=(ci == SIC - 1))
                    nc.vector.tensor_add(ob[:, hh * 512:(hh + 1) * 512], y2[:],
                                         xres_sb[:, t, hh * 512:(hh + 1) * 512])
                nc.sync.dma_start(ybase[t * 128:(t + 1) * 128, :], ob[:])

            # ---- routed blocks, 2-stage software pipeline; shared-down
            # tiles interleave between early blocks so the eviction burst
            # (8 shared adds) spreads across several block periods ----
            blk0 = blocks[0] + (0,)
            pend = (blk0, routed_gu(blk0))
            shared_down(0)
            shared_down(1)
            for bi in range(1, len(blocks)):
                blk = blocks[bi] + (bi,)
                gus = routed_gu(blk)
                routed_down(*pend)
                pend = (blk, gus)
                if bi == 1:
                    shared_down(2)
                    shared_down(3)
            routed_down(*pend)

    nc.compile()
    _BUILD_CACHE[key] = nc
    return nc


def _prepare(inputs: dict, caps, pairs, cw: np.ndarray, idx: list[np.ndarray]):
    """Build per-core input maps. idx[e] = token indices routed to expert e."""
    cea, ceb = caps
    blocks = _blocks(cea, ceb)
    nb = len(blocks)
    xf = np.asarray(inputs["hidden_states"], np.float32).reshape(N, H)
    xt_bf = np.ascontiguousarray(xf.T).astype(NP_BF16)        # [H, N]
    wg = np.asarray(inputs["Wg"])
    wu = np.asarray(inputs["Wu"])
    wd = np.asarray(inputs["Wd"])
    wsg = np.asarray(inputs["Ws_g"])
    wsu = np.asarray(inputs["Ws_u"])
    wsd = np.asarray(inputs["Ws_d"])
    bases = [0, cea]
    m = cea + ceb
    wsg_bf = np.ascontiguousarray(wsg.T).astype(NP_BF16)
    wsu_bf = np.ascontiguousarray(wsu.T).astype(NP_BF16)
    wsd_bf = np.ascontiguousarray(wsd.T).astype(NP_BF16)
    in_maps = []
    for core in range(NCORES):
        es = pairs[core]
        xg = np.zeros((H, m), NP_BF16)
        cwg = np.zeros((m,), np.float32)
        for j, e in enumerate(es):
            ne = len(idx[e])
            xg[:, bases[j]:bases[j] + ne] = xt_bf[:, idx[e]]
            cwg[bases[j]:bases[j] + ne] = cw[idx[e], e]
        xgb = np.zeros((nb, H, GB), NP_BF16)
        for i, (_, col0, n, _) in enumerate(blocks):
            xgb[i, :, :n] = xg[:, col0:col0 + n]
        sl = slice(core * NSH, (core + 1) * NSH)
        in_maps.append({
            "xg_t": xgb,
            "cw_pt": np.ascontiguousarray(cwg.reshape(-1, 128).T),
            "wg_t": np.ascontiguousarray(
                wg[list(es)].transpose(0, 2, 1)).astype(NP_BF16),
            "wu_t": np.ascontiguousarray(
                wu[list(es)].transpose(0, 2, 1)).astype(NP_BF16),
            "wd_t": np.ascontiguousarray(
                wd[list(es)].transpose(0, 2, 1)).astype(NP_BF16),
            "xs_t": np.ascontiguousarray(xt_bf[:, sl]),
            "xres": np.ascontiguousarray(xf[sl]).astype(NP_BF16),
            "wsg_t": wsg_bf,
            "wsu_t": wsu_bf,
            "wsd_t": wsd_bf,
        })
    return in_maps


def _combine(results, caps, pairs, idx: list[np.ndarray]) -> np.ndarray:
    out = np.empty((N, H), np.float32)
    bases = [0, caps[0]]
    for core in range(NCORES):
        out[core * NSH:(core + 1) * NSH] = np.asarray(
            results[core]["ybase"], np.float32)
    for core in range(NCORES):
        ygr = np.asarray(results[core]["yg"], np.float32)
        for j, e in enumerate(pairs[core]):
            ne = len(idx[e])
            out[idx[e]] += ygr[bases[j]:bases[j] + ne]
    return out.reshape(B, S, H)


def _route(inputs: dict):
    xf = np.asarray(inputs["hidden_states"], np.float32).reshape(N, H)
    cw = _gate_cw(xf, np.asarray(inputs["gate_w"], np.float32),
                  np.asarray(inputs["gate_bias"], np.float32))
    idx = [np.nonzero(cw[:, e])[0] for e in range(E)]
    loads = np.array([len(i) for i in idx])
    order = np.argsort(-loads, kind="stable")
    bigs, smalls = order[:NCORES], order[NCORES:][::-1]
    pairs = [(int(a), int(b)) for a, b in zip(bigs, smalls)]
    cea = max(MIN_CE, -(-int(loads[bigs].max()) // 128) * 128)
    ceb = max(1024, -(-int(loads[smalls].max()) // 128) * 128)
    return cw, idx, (cea, ceb), pairs


def _run(inputs: dict, trace: bool = False, tmpdir: str | None = None):
    cw, idx, caps, pairs = _route(inputs)
    nc = _build(*caps)
    in_maps = _prepare(inputs, caps, pairs, cw, idx)
    res = run_bass_kernel_spmd(nc, in_maps, list(range(NCORES)),
                               trace=trace, tmpdir=tmpdir)
    return _combine(res.results, caps, pairs, idx), res


def kernel(**inputs) -> np.ndarray:
    out, _ = _run(inputs, trace=False)
    return out


def _install_prof_shim():
    """Make run_bass_kernel_spmd(trace=True) work under axon in this image."""
    if "antenv.axon_hooks" in sys.modules:
        return
    try:
        from trn_agent_boot.trn_boot import _ntff_profile_via_ctypes
        hook = _ntff_profile_via_ctypes("/opt/axon/libaxon_pjrt.so")
    except Exception:
        hook = None
    mod = types.ModuleType("antenv.axon_hooks")
    mod.get_axon_ntff_profile_hook = lambda: hook
    mod.set_axon_ntff_profile_hook = lambda h: None
    sys.modules["antenv.axon_hooks"] = mod
    import concourse.bass_utils as bu
    bu.upload_artifacts = lambda tmpdir: tmpdir


def kernel_traced(tmpdir=None, all_cores=False, **inputs):
    """Returns (output, BassKernelResults with exec_time_ns)."""
    _install_prof_shim()
    if all_cores:
        os.environ["BASS_PERFETTO_PROFILE_ALL_CORES"] = "1"
    out, res = _run(inputs, trace=True, tmpdir=tmpdir)
    return out, res


# revision 17
# speedup vs baseline: 1.0358x; 1.0149x over previous
"""DeepseekV3 MoE (E=16, K=4, H=1024, I=512, shared 2x) on 8 trn2 NeuronCores.

Expert-parallel: 2 routed experts per core (host gathers each expert's tokens),
shared expert + residual data-parallel over 512-token slices. Host does the
gate (fp32 numpy, reference-exact) and the token all-to-all (gather/scatter);
all matmuls/activations run on-device in bf16 with fp32 accumulation.

v2: weight-stationary G/U matmuls produce G^T/U^T [I-part, tok] directly in
PSUM, so the down-proj needs no PE transposes and no act copies; the ACT
engine runs only Sigmoid (no table thrash); combine weights are folded into
the down-proj PSUM eviction on DVE; DMA pieces are contiguous SBUF ranges
ordered so the PE starts ~2us in and never waits on weights.
"""

import os
import sys
import types
import numpy as np
import ml_dtypes

import concourse.bass as bass
import concourse.mybir as mybir
import concourse.tile as tile
from concourse import bacc
from concourse.bass_utils import run_bass_kernel_spmd

BF16 = mybir.dt.bfloat16
F32 = mybir.dt.float32
NP_BF16 = ml_dtypes.bfloat16

E, K, NG, TG = 16, 4, 4, 2
SCALE = 2.5
H, I, SH_I = 1024, 512, 1024
B, S = 2, 2048
N = B * S
NCORES = 8
EPC = E // NCORES          # experts per core = 2
NSH = N // NCORES          # shared-expert tokens per core = 512
HC = H // 128              # 8 h-chunks
IC = I // 128              # 4 i-chunks (routed)
SIC = SH_I // 128          # 8 i-chunks (shared)
NQ = 4                     # shared i-quarters (2 chunks each)
MIN_CE = 1152              # per-expert token capacity (multiple of 128)
GB = 256                   # routed token-group/block width


def _gate_cw(xf: np.ndarray, gate_w: np.ndarray, gate_bias: np.ndarray) -> np.ndarray:
    """Reference-exact MoE gate in numpy fp32. Returns cw [N, E]."""
    logits = xf @ gate_w.T
    scores = 1.0 / (1.0 + np.exp(-logits))
    sfc = scores + gate_bias
    epg = E // NG
    grp = sfc.reshape(N, NG, epg)
    top2 = np.sort(grp, axis=-1)[:, :, -2:].sum(-1)
    gidx = np.argsort(-top2, axis=1, kind="stable")[:, :TG]
    gmask = np.zeros((N, NG), bool)
    np.put_along_axis(gmask, gidx, True, axis=1)
    emask = np.repeat(gmask, epg, axis=1)
    masked = np.where(emask, sfc, -np.inf)
    topk_idx = np.argsort(-masked, axis=1, kind="stable")[:, :K]
    topk_w = np.take_along_axis(scores, topk_idx, axis=1)
    topk_w = topk_w / (topk_w.sum(-1, keepdims=True) + 1e-20)
    topk_w = topk_w * SCALE
    cw = np.zeros((N, E), np.float32)
    np.put_along_axis(cw, topk_idx, topk_w.astype(np.float32), axis=1)
    return cw


def _blocks(cea: int, ceb: int):
    """Token-group blocks tiling the [m] slot space: (slot, col0, n, tt0)."""
    out = []
    tt = 0
    for e, (base, cap) in enumerate(((0, cea), (cea, ceb))):
        off = 0
        while off < cap:
            n = min(GB, cap - off)
            out.append((e, base + off, n, tt))
            tt += -(-n // 128)
            off += n
    return out


_BUILD_CACHE: dict[tuple, object] = {}


def _build(cea: int, ceb: int):
    """Build + compile the per-core SPMD Tile program."""
    key = (cea, ceb)
    if key in _BUILD_CACHE:
        return _BUILD_CACHE[key]
    blocks = _blocks(cea, ceb)
    nb = len(blocks)
    tt_total = blocks[-1][3] + -(-blocks[-1][2] // 128)
    m = cea + ceb

    nc = bacc.Bacc("TRN2", target_bir_lowering=False, debug=False,
                   num_devices=NCORES)
    xg_t = nc.dram_tensor("xg_t", [nb, H, GB], BF16, kind="ExternalInput").ap()
    cw_pt = nc.dram_tensor("cw_pt", [128, tt_total], F32, kind="ExternalInput").ap()
    wg_t = nc.dram_tensor("wg_t", [EPC, H, I], BF16, kind="ExternalInput").ap()
    wu_t = nc.dram_tensor("wu_t", [EPC, H, I], BF16, kind="ExternalInput").ap()
    wd_t = nc.dram_tensor("wd_t", [EPC, I, H], BF16, kind="ExternalInput").ap()
    xs_t = nc.dram_tensor("xs_t", [H, NSH], BF16, kind="ExternalInput").ap()
    xres = nc.dram_tensor("xres", [NSH, H], BF16, kind="ExternalInput").ap()
    wsg_t = nc.dram_tensor("wsg_t", [H, SH_I], BF16, kind="ExternalInput").ap()
    wsu_t = nc.dram_tensor("wsu_t", [H, SH_I], BF16, kind="ExternalInput").ap()
    wsd_t = nc.dram_tensor("wsd_t", [SH_I, H], BF16, kind="ExternalInput").ap()
    yg = nc.dram_tensor("yg", [m, H], BF16, kind="ExternalOutput").ap()
    ybase = nc.dram_tensor("ybase", [NSH, H], BF16, kind="ExternalOutput").ap()

    SIGM = mybir.ActivationFunctionType.Sigmoid
    COPY = mybir.ActivationFunctionType.Copy

    with tile.TileContext(nc) as tc:
        with (
            tc.tile_pool(name="const", bufs=1) as const,
            tc.tile_pool(name="sb_act", bufs=6) as sb_act,
            tc.tile_pool(name="act_rt", bufs=2) as act_rt,
            tc.tile_pool(name="sb_out", bufs=3) as sb_out,
            tc.tile_pool(name="ps_gu", bufs=5, space=bass.MemorySpace.PSUM) as ps_gu,
            tc.tile_pool(name="ps_y", bufs=3, space=bass.MemorySpace.PSUM) as ps_y,
        ):
            # ---- resident SBUF tiles; every DMA piece is a contiguous SBUF
            # byte range (Tile tracks DMA->compute deps by bounding box), in
            # the order compute consumes them ----
            wsg_sb = const.tile([128, NQ, HC, 256], BF16, tag="wsg")
            wsu_sb = const.tile([128, NQ, HC, 256], BF16, tag="wsu")
            xs_sb = const.tile([128, HC, NSH], BF16, tag="xs")
            wsd_sb = const.tile([128, SIC, H], BF16, tag="wsd")
            xres_sb = const.tile([128, NSH // 128, H], BF16, tag="xres")
            cw_sb = const.tile([128, tt_total], F32, tag="cw")
            wg_sb = const.tile([128, EPC, HC, I], BF16, tag="wg")
            wu_sb = const.tile([128, EPC, HC, I], BF16, tag="wu")
            wd_sb = const.tile([128, EPC, IC, H], BF16, tag="wd")
            xg_sb = const.tile([128, nb, HC, GB], BF16, tag="xg")
            act_sh = const.tile([128, SIC, NSH], BF16, tag="act_sh")

            wsg_r = wsg_t.rearrange("(c p) (q i) -> p q c i", p=128, q=NQ)
            wsu_r = wsu_t.rearrange("(c p) (q i) -> p q c i", p=128, q=NQ)
            xs_r = xs_t.rearrange("(c p) n -> p c n", p=128)
            wsd_r = wsd_t.rearrange("(c p) h -> p c h", p=128)
            xres_r = xres.rearrange("(t p) h -> p t h", p=128)
            wg_r = wg_t.rearrange("e (c p) i -> p e c i", p=128)
            wu_r = wu_t.rearrange("e (c p) i -> p e c i", p=128)
            wd_r = wd_t.rearrange("e (c p) h -> p e c h", p=128)
            xg_r = xg_t.rearrange("b (c p) n -> p b c n", p=128)

            # first pieces in consumption order; wsu q0 rides the idle ACT
            # queue so it doesn't delay the SP-side critical pieces
            nc.sync.dma_start(wsg_sb[:, 0], wsg_r[:, 0])
            nc.sync.dma_start(xs_sb[:, 0:4, :], xs_r[:, 0:4, :])
            nc.sync.dma_start(xs_sb[:, 4:8, :], xs_r[:, 4:8, :])
            nc.scalar.dma_start(wsu_sb[:, 0], wsu_r[:, 0])
            for q in range(1, NQ):
                nc.sync.dma_start(wsg_sb[:, q], wsg_r[:, q])
                nc.sync.dma_start(wsu_sb[:, q], wsu_r[:, q])
            nc.sync.dma_start(cw_sb[:], cw_pt[:])
            nc.sync.dma_start(wg_sb[:, 0], wg_r[:, 0])
            nc.sync.dma_start(wu_sb[:, 0], wu_r[:, 0])
            nc.sync.dma_start(xg_sb[:, 0], xg_r[:, 0])
            nc.sync.dma_start(wsd_sb[:, 0:4, :], wsd_r[:, 0:4, :])
            nc.sync.dma_start(wsd_sb[:, 4:8, :], wsd_r[:, 4:8, :])
            nc.sync.dma_start(xres_sb[:], xres_r[:])
            nc.sync.dma_start(wd_sb[:, 0], wd_r[:, 0])
            for b in range(1, 3):
                nc.sync.dma_start(xg_sb[:, b], xg_r[:, b])
            nc.sync.dma_start(wg_sb[:, 1], wg_r[:, 1])
            nc.sync.dma_start(wu_sb[:, 1], wu_r[:, 1])
            for b in range(3, nb):
                nc.sync.dma_start(xg_sb[:, b], xg_r[:, b])
            nc.sync.dma_start(wd_sb[:, 1], wd_r[:, 1])

            # ---- shared expert G^T/U^T: out [I-local 128, tok 512] ----
            for q in range(NQ):
                g_ps, u_ps = [], []
                for w_sb, dst in ((wsg_sb, g_ps), (wsu_sb, u_ps)):
                    for i in range(2):
                        t_ps = ps_gu.tile([128, NSH], F32, tag="gu")
                        for c in range(HC):
                            nc.tensor.matmul(t_ps[:],
                                             w_sb[:, q, c, i * 128:(i + 1) * 128],
                                             xs_sb[:, c, :],
                                             start=(c == 0), stop=(c == HC - 1))
                        dst.append(t_ps)
                for i in range(2):
                    p_sb = sb_act.tile([128, NSH], BF16, tag="p")
                    nc.scalar.activation(p_sb[:], g_ps[i][:], SIGM)
                    t_sb = sb_act.tile([128, NSH], BF16, tag="t")
                    nc.vector.tensor_mul(t_sb[:], p_sb[:], g_ps[i][:])
                    nc.vector.tensor_mul(act_sh[:, 2 * q + i, :], t_sb[:],
                                         u_ps[i][:])

            def routed_gu(blk):
                e, col0, n, tt0, b = blk
                gus = []
                for w_sb in (wg_sb, wu_sb):
                    for pr in range(2):
                        t_ps = ps_gu.tile([128, 2, n], F32, tag="gu")
                        for ii in range(2):
                            ci = 2 * pr + ii
                            for c in range(HC):
                                nc.tensor.matmul(
                                    t_ps[:, ii, :],
                                    w_sb[:, e, c, ci * 128:(ci + 1) * 128],
                                    xg_sb[:, b, c, 0:n],
                                    start=(c == 0), stop=(c == HC - 1))
                        gus.append(t_ps)
                return gus

            def routed_down(blk, gus):
                e, col0, n, tt0, b = blk
                act = act_rt.tile([128, IC, n], BF16, tag="act")
                for pr in range(2):
                    g_ps, u_ps = gus[pr], gus[2 + pr]
                    p_sb = sb_act.tile([128, 2, n], BF16, tag="p")
                    nc.scalar.activation(p_sb[:], g_ps[:], SIGM)
                    t_sb = sb_act.tile([128, 2, n], BF16, tag="t")
                    nc.vector.tensor_mul(t_sb[:], p_sb[:], g_ps[:])
                    nc.vector.tensor_mul(act[:, 2 * pr:2 * pr + 2, :], t_sb[:],
                                         u_ps[:])
                last = (col0 + n == cea + ceb)
                for t in range(-(-n // 128)):
                    tt = tt0 + t
                    tk = min(128, n - t * 128)
                    r0 = col0 + t * 128
                    y_sb = sb_out.tile([128, H], BF16, tag="y")
                    for hh in range(2):
                        y_ps = ps_y.tile([128, 512], F32, tag="y_ps")
                        for ci in range(IC):
                            nc.tensor.matmul(
                                y_ps[0:tk, :],
                                act[:, ci, t * 128:t * 128 + tk],
                                wd_sb[:, e, ci, hh * 512:(hh + 1) * 512],
                                start=(ci == 0), stop=(ci == IC - 1))
                        # evictions alternate ACT/DVE so neither engine's
                        # backlog gates PSUM-bank reuse
                        if hh == 0:
                            nc.scalar.activation(
                                y_sb[0:tk, 0:512], y_ps[0:tk, :], COPY,
                                scale=cw_sb[0:tk, tt:tt + 1])
                        else:
                            nc.vector.tensor_scalar_mul(
                                y_sb[0:tk, 512:1024], y_ps[0:tk, :],
                                cw_sb[0:tk, tt:tt + 1])
                        if last:
                            nc.sync.dma_start(
                                yg[r0:r0 + tk, hh * 512:(hh + 1) * 512],
                                y_sb[0:tk, hh * 512:(hh + 1) * 512])
                    if not last:
                        nc.sync.dma_start(yg[r0:r0 + tk, :], y_sb[0:tk, :])

            def shared_down(t):
                ob = sb_out.tile([128, H], BF16, tag="ob")
                for hh in range(2):
                    y2 = ps_y.tile([128, 512], F32, tag="y_ps")
                    for ci in range(SIC):
                        nc.tensor.matmul(
                            y2[:], act_sh[:, ci, t * 128:(t + 1) * 128],
                            wsd_sb[:, ci, hh * 512:(hh + 1) * 512],
                            start=(ci == 0), stop=(ci == SIC - 1))
                    nc.vector.tensor_add(ob[:, hh * 512:(hh + 1) * 512], y2[:],
                                         xres_sb[:, t, hh * 512:(hh + 1) * 512])
                nc.sync.dma_start(ybase[t * 128:(t + 1) * 128, :], ob[:])

            # ---- routed blocks, 2-stage software pipeline; shared-down
            # tiles interleave between early blocks so the eviction burst
            # (8 shared adds) spreads across several block periods ----
            blk0 = blocks[0] + (0,)
            pend = (blk0, routed_gu(blk0))
            shared_down(0)
            shared_down(1)
            for bi in range(1, len(blocks)):
                blk = blocks[bi] + (bi,)
                gus = routed_gu(blk)
                routed_down(*pend)
                pend = (blk, gus)
                if bi == 1:
                    shared_down(2)
                    shared_down(3)
            routed_down(*pend)

    nc.compile()
    _BUILD_CACHE[key] = nc
    return nc


def _prepare(inputs: dict, caps, pairs, cw: np.ndarray, idx: list[np.ndarray]):
    """Build per-core input maps. idx[e] = token indices routed to expert e."""
    cea, ceb = caps
    blocks = _blocks(cea, ceb)
    nb = len(blocks)
    xf = np.asarray(inputs["hidden_states"], np.float32).reshape(N, H)
    xt_bf = np.ascontiguousarray(xf.T).astype(NP_BF16)        # [H, N]
    wg = np.asarray(inputs["Wg"])
    wu = np.asarray(inputs["Wu"])
    wd = np.asarray(inputs["Wd"])
    wsg = np.asarray(inputs["Ws_g"])
    wsu = np.asarray(inputs["Ws_u"])
    wsd = np.asarray(inputs["Ws_d"])
    bases = [0, cea]
    m = cea + ceb
    wsg_bf = np.ascontiguousarray(wsg.T).astype(NP_BF16)
    wsu_bf = np.ascontiguousarray(wsu.T).astype(NP_BF16)
    wsd_bf = np.ascontiguousarray(wsd.T).astype(NP_BF16)
    in_maps = []
    for core in range(NCORES):
        es = pairs[core]
        xg = np.zeros((H, m), NP_BF16)
        cwg = np.zeros((m,), np.float32)
        for j, e in enumerate(es):
            ne = len(idx[e])
            xg[:, bases[j]:bases[j] + ne] = xt_bf[:, idx[e]]
            cwg[bases[j]:bases[j] + ne] = cw[idx[e], e]
        xgb = np.zeros((nb, H, GB), NP_BF16)
        for i, (_, col0, n, _) in enumerate(blocks):
            xgb[i, :, :n] = xg[:, col0:col0 + n]
        tt_total = blocks[-1][3] + -(-blocks[-1][2] // 128)
        cw_bt = np.zeros((128, tt_total), np.float32)
        for _, col0, n, tt0 in blocks:
            for t in range(-(-n // 128)):
                tk = min(128, n - t * 128)
                cw_bt[:tk, tt0 + t] = cwg[col0 + t * 128:col0 + t * 128 + tk]
        sl = slice(core * NSH, (core + 1) * NSH)
        in_maps.append({
            "xg_t": xgb,
            "cw_pt": cw_bt,
            "wg_t": np.ascontiguousarray(
                wg[list(es)].transpose(0, 2, 1)).astype(NP_BF16),
            "wu_t": np.ascontiguousarray(
                wu[list(es)].transpose(0, 2, 1)).astype(NP_BF16),
            "wd_t": np.ascontiguousarray(
                wd[list(es)].transpose(0, 2, 1)).astype(NP_BF16),
            "xs_t": np.ascontiguousarray(xt_bf[:, sl]),
            "xres": np.ascontiguousarray(xf[sl]).astype(NP_BF16),
            "wsg_t": wsg_bf,
            "wsu_t": wsu_bf,
            "wsd_t": wsd_bf,
        })
    return in_maps


def _combine(results, caps, pairs, idx: list[np.ndarray]) -> np.ndarray:
    out = np.empty((N, H), np.float32)
    bases = [0, caps[0]]
    for core in range(NCORES):
        out[core * NSH:(core + 1) * NSH] = np.asarray(
            results[core]["ybase"], np.float32)
    for core in range(NCORES):
        ygr = np.asarray(results[core]["yg"], np.float32)
        for j, e in enumerate(pairs[core]):
            ne = len(idx[e])
            out[idx[e]] += ygr[bases[j]:bases[j] + ne]
    return out.reshape(B, S, H)


def _route(inputs: dict):
    xf = np.asarray(inputs["hidden_states"], np.float32).reshape(N, H)
    cw = _gate_cw(xf, np.asarray(inputs["gate_w"], np.float32),
                  np.asarray(inputs["gate_bias"], np.float32))
    idx = [np.nonzero(cw[:, e])[0] for e in range(E)]
    loads = np.array([len(i) for i in idx])
    order = np.argsort(-loads, kind="stable")
    bigs, smalls = order[:NCORES], order[NCORES:][::-1]
    pairs = [(int(a), int(b)) for a, b in zip(bigs, smalls)]
    # capacity = max slot load rounded up to 32; tail tiles beyond the last
    # full 256-block run at reduced matmul free-dim (mostly padding otherwise)
    cea = max(32, -(-int(loads[bigs].max()) // 32) * 32)
    ceb = max(32, -(-int(loads[smalls].max()) // 32) * 32)
    return cw, idx, (cea, ceb), pairs


def _run(inputs: dict, trace: bool = False, tmpdir: str | None = None):
    cw, idx, caps, pairs = _route(inputs)
    nc = _build(*caps)
    in_maps = _prepare(inputs, caps, pairs, cw, idx)
    res = run_bass_kernel_spmd(nc, in_maps, list(range(NCORES)),
                               trace=trace, tmpdir=tmpdir)
    return _combine(res.results, caps, pairs, idx), res


def kernel(**inputs) -> np.ndarray:
    out, _ = _run(inputs, trace=False)
    return out


def _install_prof_shim():
    """Make run_bass_kernel_spmd(trace=True) work under axon in this image."""
    if "antenv.axon_hooks" in sys.modules:
        return
    try:
        from trn_agent_boot.trn_boot import _ntff_profile_via_ctypes
        hook = _ntff_profile_via_ctypes("/opt/axon/libaxon_pjrt.so")
    except Exception:
        hook = None
    mod = types.ModuleType("antenv.axon_hooks")
    mod.get_axon_ntff_profile_hook = lambda: hook
    mod.set_axon_ntff_profile_hook = lambda h: None
    sys.modules["antenv.axon_hooks"] = mod
    import concourse.bass_utils as bu
    bu.upload_artifacts = lambda tmpdir: tmpdir


def kernel_traced(tmpdir=None, all_cores=False, **inputs):
    """Returns (output, BassKernelResults with exec_time_ns)."""
    _install_prof_shim()
    if all_cores:
        os.environ["BASS_PERFETTO_PROFILE_ALL_CORES"] = "1"
    out, res = _run(inputs, trace=True, tmpdir=tmpdir)
    return out, res


# revision 18
# speedup vs baseline: 1.0492x; 1.0129x over previous
"""DeepseekV3 MoE (E=16, K=4, H=1024, I=512, shared 2x) on 8 trn2 NeuronCores.

Expert-parallel: 2 routed experts per core (host gathers each expert's tokens),
shared expert + residual data-parallel over 512-token slices. Host does the
gate (fp32 numpy, reference-exact) and the token all-to-all (gather/scatter);
all matmuls/activations run on-device in bf16 with fp32 accumulation.

v2: weight-stationary G/U matmuls produce G^T/U^T [I-part, tok] directly in
PSUM, so the down-proj needs no PE transposes and no act copies; the ACT
engine runs only Sigmoid (no table thrash); combine weights are folded into
the down-proj PSUM eviction on DVE; DMA pieces are contiguous SBUF ranges
ordered so the PE starts ~2us in and never waits on weights.
"""

import os
import sys
import types
import numpy as np
import ml_dtypes

import concourse.bass as bass
import concourse.mybir as mybir
import concourse.tile as tile
from concourse import bacc
from concourse.bass_utils import run_bass_kernel_spmd

BF16 = mybir.dt.bfloat16
F32 = mybir.dt.float32
NP_BF16 = ml_dtypes.bfloat16

E, K, NG, TG = 16, 4, 4, 2
SCALE = 2.5
H, I, SH_I = 1024, 512, 1024
B, S = 2, 2048
N = B * S
NCORES = 8
EPC = E // NCORES          # experts per core = 2
NSH = N // NCORES          # shared-expert tokens per core = 512
HC = H // 128              # 8 h-chunks
IC = I // 128              # 4 i-chunks (routed)
SIC = SH_I // 128          # 8 i-chunks (shared)
NQ = 4                     # shared i-quarters (2 chunks each)
MIN_CE = 1152              # per-expert token capacity (multiple of 128)
GB = 256                   # routed token-group/block width


def _gate_cw(xf: np.ndarray, gate_w: np.ndarray, gate_bias: np.ndarray) -> np.ndarray:
    """Reference-exact MoE gate in numpy fp32. Returns cw [N, E]."""
    logits = xf @ gate_w.T
    scores = 1.0 / (1.0 + np.exp(-logits))
    sfc = scores + gate_bias
    epg = E // NG
    grp = sfc.reshape(N, NG, epg)
    top2 = np.sort(grp, axis=-1)[:, :, -2:].sum(-1)
    gidx = np.argsort(-top2, axis=1, kind="stable")[:, :TG]
    gmask = np.zeros((N, NG), bool)
    np.put_along_axis(gmask, gidx, True, axis=1)
    emask = np.repeat(gmask, epg, axis=1)
    masked = np.where(emask, sfc, -np.inf)
    topk_idx = np.argsort(-masked, axis=1, kind="stable")[:, :K]
    topk_w = np.take_along_axis(scores, topk_idx, axis=1)
    topk_w = topk_w / (topk_w.sum(-1, keepdims=True) + 1e-20)
    topk_w = topk_w * SCALE
    cw = np.zeros((N, E), np.float32)
    np.put_along_axis(cw, topk_idx, topk_w.astype(np.float32), axis=1)
    return cw


def _blocks(cea: int, ceb: int):
    """Token-group blocks tiling the [m] slot space: (slot, col0, n, tt0)."""
    out = []
    tt = 0
    for e, (base, cap) in enumerate(((0, cea), (cea, ceb))):
        off = 0
        while off < cap:
            n = min(GB, cap - off)
            out.append((e, base + off, n, tt))
            tt += -(-n // 128)
            off += n
    return out


_BUILD_CACHE: dict[tuple, object] = {}


def _build(cea: int, ceb: int):
    """Build + compile the per-core SPMD Tile program."""
    key = (cea, ceb)
    if key in _BUILD_CACHE:
        return _BUILD_CACHE[key]
    blocks = _blocks(cea, ceb)
    nb = len(blocks)
    tt_total = blocks[-1][3] + -(-blocks[-1][2] // 128)
    m = cea + ceb

    nc = bacc.Bacc("TRN2", target_bir_lowering=False, debug=False,
                   num_devices=NCORES)
    xg_t = nc.dram_tensor("xg_t", [nb, H, GB], BF16, kind="ExternalInput").ap()
    cw_pt = nc.dram_tensor("cw_pt", [128, tt_total], F32, kind="ExternalInput").ap()
    wg_t = nc.dram_tensor("wg_t", [EPC, H, I], BF16, kind="ExternalInput").ap()
    wu_t = nc.dram_tensor("wu_t", [EPC, H, I], BF16, kind="ExternalInput").ap()
    wd_t = nc.dram_tensor("wd_t", [EPC, I, H], BF16, kind="ExternalInput").ap()
    xs_t = nc.dram_tensor("xs_t", [H, NSH], BF16, kind="ExternalInput").ap()
    xres = nc.dram_tensor("xres", [NSH, H], BF16, kind="ExternalInput").ap()
    wsg_t = nc.dram_tensor("wsg_t", [H, SH_I], BF16, kind="ExternalInput").ap()
    wsu_t = nc.dram_tensor("wsu_t", [H, SH_I], BF16, kind="ExternalInput").ap()
    wsd_t = nc.dram_tensor("wsd_t", [SH_I, H], BF16, kind="ExternalInput").ap()
    yg = nc.dram_tensor("yg", [m, H], BF16, kind="ExternalOutput").ap()
    ybase = nc.dram_tensor("ybase", [NSH, H], BF16, kind="ExternalOutput").ap()

    SIGM = mybir.ActivationFunctionType.Sigmoid
    COPY = mybir.ActivationFunctionType.Copy

    with tile.TileContext(nc) as tc:
        with (
            tc.tile_pool(name="const", bufs=1) as const,
            tc.tile_pool(name="sb_act", bufs=6) as sb_act,
            tc.tile_pool(name="act_rt", bufs=2) as act_rt,
            tc.tile_pool(name="sb_out", bufs=3) as sb_out,
            tc.tile_pool(name="ps_gu", bufs=5, space=bass.MemorySpace.PSUM) as ps_gu,
            tc.tile_pool(name="ps_y", bufs=3, space=bass.MemorySpace.PSUM) as ps_y,
        ):
            # ---- resident SBUF tiles; every DMA piece is a contiguous SBUF
            # byte range (Tile tracks DMA->compute deps by bounding box), in
            # the order compute consumes them ----
            wsg_sb = const.tile([128, NQ, HC, 256], BF16, tag="wsg")
            wsu_sb = const.tile([128, NQ, HC, 256], BF16, tag="wsu")
            xs_sb = const.tile([128, HC, NSH], BF16, tag="xs")
            wsd_sb = const.tile([128, SIC, H], BF16, tag="wsd")
            xres_sb = const.tile([128, NSH // 128, H], BF16, tag="xres")
            cw_sb = const.tile([128, tt_total], F32, tag="cw")
            wg_sb = const.tile([128, EPC, HC, I], BF16, tag="wg")
            wu_sb = const.tile([128, EPC, HC, I], BF16, tag="wu")
            wd_sb = const.tile([128, EPC, IC, H], BF16, tag="wd")
            xg_sb = const.tile([128, nb, HC, GB], BF16, tag="xg")
            act_sh = const.tile([128, SIC, NSH], BF16, tag="act_sh")

            wsg_r = wsg_t.rearrange("(c p) (q i) -> p q c i", p=128, q=NQ)
            wsu_r = wsu_t.rearrange("(c p) (q i) -> p q c i", p=128, q=NQ)
            xs_r = xs_t.rearrange("(c p) n -> p c n", p=128)
            wsd_r = wsd_t.rearrange("(c p) h -> p c h", p=128)
            xres_r = xres.rearrange("(t p) h -> p t h", p=128)
            wg_r = wg_t.rearrange("e (c p) i -> p e c i", p=128)
            wu_r = wu_t.rearrange("e (c p) i -> p e c i", p=128)
            wd_r = wd_t.rearrange("e (c p) h -> p e c h", p=128)
            xg_r = xg_t.rearrange("b (c p) n -> p b c n", p=128)

            # first pieces in consumption order, sized so the first matmul
            # can start after ~0.5 MB; wsu q0 rides the idle ACT queue so it
            # doesn't delay the SP-side critical pieces
            nc.sync.dma_start(wsg_sb[:, 0, 0:4, :], wsg_r[:, 0, 0:4, :])
            nc.sync.dma_start(xs_sb[:, 0:2, :], xs_r[:, 0:2, :])
            nc.sync.dma_start(xs_sb[:, 2:4, :], xs_r[:, 2:4, :])
            nc.sync.dma_start(wsg_sb[:, 0, 4:8, :], wsg_r[:, 0, 4:8, :])
            nc.sync.dma_start(xs_sb[:, 4:6, :], xs_r[:, 4:6, :])
            nc.sync.dma_start(xs_sb[:, 6:8, :], xs_r[:, 6:8, :])
            nc.scalar.dma_start(wsu_sb[:, 0], wsu_r[:, 0])
            for q in range(1, NQ):
                nc.sync.dma_start(wsg_sb[:, q], wsg_r[:, q])
                nc.sync.dma_start(wsu_sb[:, q], wsu_r[:, q])
            nc.sync.dma_start(cw_sb[:], cw_pt[:])
            nc.sync.dma_start(wg_sb[:, 0], wg_r[:, 0])
            nc.sync.dma_start(wu_sb[:, 0], wu_r[:, 0])
            nc.sync.dma_start(xg_sb[:, 0], xg_r[:, 0])
            nc.sync.dma_start(wsd_sb[:, 0:4, :], wsd_r[:, 0:4, :])
            nc.sync.dma_start(wsd_sb[:, 4:8, :], wsd_r[:, 4:8, :])
            nc.sync.dma_start(xres_sb[:], xres_r[:])
            nc.sync.dma_start(wd_sb[:, 0], wd_r[:, 0])
            for b in range(1, 3):
                nc.sync.dma_start(xg_sb[:, b], xg_r[:, b])
            nc.sync.dma_start(wg_sb[:, 1], wg_r[:, 1])
            nc.sync.dma_start(wu_sb[:, 1], wu_r[:, 1])
            for b in range(3, nb):
                nc.sync.dma_start(xg_sb[:, b], xg_r[:, b])
            nc.sync.dma_start(wd_sb[:, 1], wd_r[:, 1])

            # ---- shared expert G^T/U^T: out [I-local 128, tok 512] ----
            for q in range(NQ):
                g_ps, u_ps = [], []
                for w_sb, dst in ((wsg_sb, g_ps), (wsu_sb, u_ps)):
                    for i in range(2):
                        t_ps = ps_gu.tile([128, NSH], F32, tag="gu")
                        for c in range(HC):
                            nc.tensor.matmul(t_ps[:],
                                             w_sb[:, q, c, i * 128:(i + 1) * 128],
                                             xs_sb[:, c, :],
                                             start=(c == 0), stop=(c == HC - 1))
                        dst.append(t_ps)
                for i in range(2):
                    p_sb = sb_act.tile([128, NSH], BF16, tag="p")
                    nc.scalar.activation(p_sb[:], g_ps[i][:], SIGM)
                    t_sb = sb_act.tile([128, NSH], BF16, tag="t")
                    nc.vector.tensor_mul(t_sb[:], p_sb[:], g_ps[i][:])
                    nc.vector.tensor_mul(act_sh[:, 2 * q + i, :], t_sb[:],
                                         u_ps[i][:])

            def routed_gu(blk):
                e, col0, n, tt0, b = blk
                gus = []
                for w_sb in (wg_sb, wu_sb):
                    for pr in range(2):
                        t_ps = ps_gu.tile([128, 2, n], F32, tag="gu")
                        for ii in range(2):
                            ci = 2 * pr + ii
                            for c in range(HC):
                                nc.tensor.matmul(
                                    t_ps[:, ii, :],
                                    w_sb[:, e, c, ci * 128:(ci + 1) * 128],
                                    xg_sb[:, b, c, 0:n],
                                    start=(c == 0), stop=(c == HC - 1))
                        gus.append(t_ps)
                return gus

            def routed_down(blk, gus):
                e, col0, n, tt0, b = blk
                act = act_rt.tile([128, IC, n], BF16, tag="act")
                for pr in range(2):
                    g_ps, u_ps = gus[pr], gus[2 + pr]
                    p_sb = sb_act.tile([128, 2, n], BF16, tag="p")
                    nc.scalar.activation(p_sb[:], g_ps[:], SIGM)
                    t_sb = sb_act.tile([128, 2, n], BF16, tag="t")
                    nc.vector.tensor_mul(t_sb[:], p_sb[:], g_ps[:])
                    nc.vector.tensor_mul(act[:, 2 * pr:2 * pr + 2, :], t_sb[:],
                                         u_ps[:])
                last = (col0 + n == cea + ceb)
                for t in range(-(-n // 128)):
                    tt = tt0 + t
                    tk = min(128, n - t * 128)
                    r0 = col0 + t * 128
                    y_sb = sb_out.tile([128, H], BF16, tag="y")
                    for hh in range(2):
                        y_ps = ps_y.tile([128, 512], F32, tag="y_ps")
                        for ci in range(IC):
                            nc.tensor.matmul(
                                y_ps[0:tk, :],
                                act[:, ci, t * 128:t * 128 + tk],
                                wd_sb[:, e, ci, hh * 512:(hh + 1) * 512],
                                start=(ci == 0), stop=(ci == IC - 1))
                        # evictions alternate ACT/DVE so neither engine's
                        # backlog gates PSUM-bank reuse
                        if hh == 0:
                            nc.scalar.activation(
                                y_sb[0:tk, 0:512], y_ps[0:tk, :], COPY,
                                scale=cw_sb[0:tk, tt:tt + 1])
                        else:
                            nc.vector.tensor_scalar_mul(
                                y_sb[0:tk, 512:1024], y_ps[0:tk, :],
                                cw_sb[0:tk, tt:tt + 1])
                        if last:
                            nc.sync.dma_start(
                                yg[r0:r0 + tk, hh * 512:(hh + 1) * 512],
                                y_sb[0:tk, hh * 512:(hh + 1) * 512])
                    if not last:
                        nc.sync.dma_start(yg[r0:r0 + tk, :], y_sb[0:tk, :])

            def shared_down(t):
                ob = sb_out.tile([128, H], BF16, tag="ob")
                for hh in range(2):
                    y2 = ps_y.tile([128, 512], F32, tag="y_ps")
                    for ci in range(SIC):
                        nc.tensor.matmul(
                            y2[:], act_sh[:, ci, t * 128:(t + 1) * 128],
                            wsd_sb[:, ci, hh * 512:(hh + 1) * 512],
                            start=(ci == 0), stop=(ci == SIC - 1))
                    nc.vector.tensor_add(ob[:, hh * 512:(hh + 1) * 512], y2[:],
                                         xres_sb[:, t, hh * 512:(hh + 1) * 512])
                nc.sync.dma_start(ybase[t * 128:(t + 1) * 128, :], ob[:])

            # ---- routed blocks, 2-stage software pipeline; shared-down
            # tiles interleave between early blocks so the eviction burst
            # (8 shared adds) spreads across several block periods ----
            blk0 = blocks[0] + (0,)
            pend = (blk0, routed_gu(blk0))
            shared_down(0)
            shared_down(1)
            for bi in range(1, len(blocks)):
                blk = blocks[bi] + (bi,)
                gus = routed_gu(blk)
                routed_down(*pend)
                pend = (blk, gus)
                if bi == 1:
                    shared_down(2)
                    shared_down(3)
            routed_down(*pend)

    nc.compile()
    _BUILD_CACHE[key] = nc
    return nc


def _prepare(inputs: dict, caps, pairs, cw: np.ndarray, idx: list[np.ndarray]):
    """Build per-core input maps. idx[e] = token indices routed to expert e."""
    cea, ceb = caps
    blocks = _blocks(cea, ceb)
    nb = len(blocks)
    xf = np.asarray(inputs["hidden_states"], np.float32).reshape(N, H)
    xt_bf = np.ascontiguousarray(xf.T).astype(NP_BF16)        # [H, N]
    wg = np.asarray(inputs["Wg"])
    wu = np.asarray(inputs["Wu"])
    wd = np.asarray(inputs["Wd"])
    wsg = np.asarray(inputs["Ws_g"])
    wsu = np.asarray(inputs["Ws_u"])
    wsd = np.asarray(inputs["Ws_d"])
    bases = [0, cea]
    m = cea + ceb
    wsg_bf = np.ascontiguousarray(wsg.T).astype(NP_BF16)
    wsu_bf = np.ascontiguousarray(wsu.T).astype(NP_BF16)
    wsd_bf = np.ascontiguousarray(wsd.T).astype(NP_BF16)
    in_maps = []
    for core in range(NCORES):
        es = pairs[core]
        xg = np.zeros((H, m), NP_BF16)
        cwg = np.zeros((m,), np.float32)
        for j, e in enumerate(es):
            ne = len(idx[e])
            xg[:, bases[j]:bases[j] + ne] = xt_bf[:, idx[e]]
            cwg[bases[j]:bases[j] + ne] = cw[idx[e], e]
        xgb = np.zeros((nb, H, GB), NP_BF16)
        for i, (_, col0, n, _) in enumerate(blocks):
            xgb[i, :, :n] = xg[:, col0:col0 + n]
        tt_total = blocks[-1][3] + -(-blocks[-1][2] // 128)
        cw_bt = np.zeros((128, tt_total), np.float32)
        for _, col0, n, tt0 in blocks:
            for t in range(-(-n // 128)):
                tk = min(128, n - t * 128)
                cw_bt[:tk, tt0 + t] = cwg[col0 + t * 128:col0 + t * 128 + tk]
        sl = slice(core * NSH, (core + 1) * NSH)
        in_maps.append({
            "xg_t": xgb,
            "cw_pt": cw_bt,
            "wg_t": np.ascontiguousarray(
                wg[list(es)].transpose(0, 2, 1)).astype(NP_BF16),
            "wu_t": np.ascontiguousarray(
                wu[list(es)].transpose(0, 2, 1)).astype(NP_BF16),
            "wd_t": np.ascontiguousarray(
                wd[list(es)].transpose(0, 2, 1)).astype(NP_BF16),
            "xs_t": np.ascontiguousarray(xt_bf[:, sl]),
            "xres": np.ascontiguousarray(xf[sl]).astype(NP_BF16),
            "wsg_t": wsg_bf,
            "wsu_t": wsu_bf,
            "wsd_t": wsd_bf,
        })
    return in_maps


def _combine(results, caps, pairs, idx: list[np.ndarray]) -> np.ndarray:
    out = np.empty((N, H), np.float32)
    bases = [0, caps[0]]
    for core in range(NCORES):
        out[core * NSH:(core + 1) * NSH] = np.asarray(
            results[core]["ybase"], np.float32)
    for core in range(NCORES):
        ygr = np.asarray(results[core]["yg"], np.float32)
        for j, e in enumerate(pairs[core]):
            ne = len(idx[e])
            out[idx[e]] += ygr[bases[j]:bases[j] + ne]
    return out.reshape(B, S, H)


def _route(inputs: dict):
    xf = np.asarray(inputs["hidden_states"], np.float32).reshape(N, H)
    cw = _gate_cw(xf, np.asarray(inputs["gate_w"], np.float32),
                  np.asarray(inputs["gate_bias"], np.float32))
    idx = [np.nonzero(cw[:, e])[0] for e in range(E)]
    loads = np.array([len(i) for i in idx])
    order = np.argsort(-loads, kind="stable")
    bigs, smalls = order[:NCORES], order[NCORES:][::-1]
    pairs = [(int(a), int(b)) for a, b in zip(bigs, smalls)]
    # capacity = max slot load rounded up to 32; tail tiles beyond the last
    # full 256-block run at reduced matmul free-dim (mostly padding otherwise)
    cea = max(32, -(-int(loads[bigs].max()) // 32) * 32)
    ceb = max(32, -(-int(loads[smalls].max()) // 32) * 32)
    return cw, idx, (cea, ceb), pairs


def _run(inputs: dict, trace: bool = False, tmpdir: str | None = None):
    cw, idx, caps, pairs = _route(inputs)
    nc = _build(*caps)
    in_maps = _prepare(inputs, caps, pairs, cw, idx)
    res = run_bass_kernel_spmd(nc, in_maps, list(range(NCORES)),
                               trace=trace, tmpdir=tmpdir)
    return _combine(res.results, caps, pairs, idx), res


def kernel(**inputs) -> np.ndarray:
    out, _ = _run(inputs, trace=False)
    return out


def _install_prof_shim():
    """Make run_bass_kernel_spmd(trace=True) work under axon in this image."""
    if "antenv.axon_hooks" in sys.modules:
        return
    try:
        from trn_agent_boot.trn_boot import _ntff_profile_via_ctypes
        hook = _ntff_profile_via_ctypes("/opt/axon/libaxon_pjrt.so")
    except Exception:
        hook = None
    mod = types.ModuleType("antenv.axon_hooks")
    mod.get_axon_ntff_profile_hook = lambda: hook
    mod.set_axon_ntff_profile_hook = lambda h: None
    sys.modules["antenv.axon_hooks"] = mod
    import concourse.bass_utils as bu
    bu.upload_artifacts = lambda tmpdir: tmpdir


def kernel_traced(tmpdir=None, all_cores=False, **inputs):
    """Returns (output, BassKernelResults with exec_time_ns)."""
    _install_prof_shim()
    if all_cores:
        os.environ["BASS_PERFETTO_PROFILE_ALL_CORES"] = "1"
    out, res = _run(inputs, trace=True, tmpdir=tmpdir)
    return out, res
